# revision 14
# baseline (speedup 1.0000x reference)
"""Multi-head attention + residual + LayerNorm TRN2 Bass kernel.

Problem: B=8, S=1024, d_model=512, 16 heads x d_k=128.
Returns (out, attn) like the reference:
    out  (8, 1024, 512)  f32   layernorm(context @ W_fc + input_Q)
    attn (8, 16, 1024, 1024) f32  softmax probabilities

Sharding: data-parallel over batch, one batch element per NeuronCore (8 cores).

Per-core design (fp16 matmul inputs / f32 PSUM accumulate), k-major only:
  - host pre-transposes X inputs to (512,1024) fp16; 1/sqrt(128) folded into W_Q
  - V = X_v @ W_V once in (seq, 2048) layout
  - per head h: qT_h/kT_h = (W^T X^T) tiles (d_k on partitions)
    for each k-tile: S^T = K_kt Q^T -> exp -> mask-mul -> E^T_m, which feeds
      * context matmuls C^T_h += V_kt^T E^T_m (PSUM accumulation)
      * ones-matmuls: row sums = 1^T E^T_m (PSUM accumulation, M=1)
      * after reciprocals arrive: P^T = E^T_m * (1/rowsum) -> attnT out (fp16)
    reciprocal broadcast across partitions via a tiny DRAM round-trip
    C^T_h scaled by 1/rowsum and stashed to DRAM as fp16
  - fc: O = C^T.T @ W_fc over all 16 head blocks + residual, then LayerNorm
  - host: attn = attnT.swapaxes(-1,-2).astype(f32)  (part of unsharding)
"""

import os
import sys

for _p in ("/opt/trn_rl_repo", "/root/.axon_site/_ro/trn_rl_repo"):
    if os.path.isdir(_p) and _p not in sys.path:
        sys.path.append(_p)

import numpy as np

import concourse.bass as bass
import concourse.bacc as bacc
import concourse.tile as tile
import concourse.mybir as mybir
from concourse import bass_utils

F32 = mybir.dt.float32
F16 = mybir.dt.float16
AF = mybir.ActivationFunctionType
ALU = mybir.AluOpType

S = 1024
DM = 512
H = 16
DK = 128
NQT = S // 128  # 8 k/q tiles of 128
EPS = 1e-5


def build_kernel(n_cores: int = 8):
    nc = bacc.Bacc(
        "TRN2",
        target_bir_lowering=False,
        debug=False,
        enable_asserts=False,
        num_devices=n_cores,
    )

    xqT = nc.dram_tensor("xqT", (DM, S), F16, kind="ExternalInput")
    xkT = nc.dram_tensor("xkT", (DM, S), F16, kind="ExternalInput")
    xvT = nc.dram_tensor("xvT", (DM, S), F16, kind="ExternalInput")
    xq = nc.dram_tensor("xq", (S, DM), F32, kind="ExternalInput")
    wq = nc.dram_tensor("wq", (DM, H * DK), F16, kind="ExternalInput")
    wk = nc.dram_tensor("wk", (DM, H * DK), F16, kind="ExternalInput")
    wv = nc.dram_tensor("wv", (DM, H * DK), F16, kind="ExternalInput")
    wfc = nc.dram_tensor("wfc", (H * DK, DM), F16, kind="ExternalInput")
    m01T = nc.dram_tensor("m01T", (S, S), F16, kind="ExternalInput")

    attnT = nc.dram_tensor("attnT", (H, S, S), F16, kind="ExternalOutput")
    outp = nc.dram_tensor("outp", (S, DM), F32, kind="ExternalOutput")

    with tile.TileContext(nc) as tc:
        _body(nc, tc, xqT, xkT, xvT, xq, wq, wk, wv, wfc, m01T, attnT, outp)

    nc.compile()
    return nc


def _bcast_ap(row_ap, parts, free_dims):
    """Partition-broadcast read AP over a DRAM row."""
    return bass.AP(
        tensor=row_ap.tensor,
        offset=row_ap.offset,
        ap=[[0, parts]] + free_dims,
    )


def _body(nc, tc, xqT, xkT, xvT, xq, wq, wk, wv, wfc, m01T, attnT, outp):
    from contextlib import ExitStack

    with ExitStack() as ctx:
        consts = ctx.enter_context(tc.tile_pool(name="consts", bufs=1))
        dram = ctx.enter_context(tc.tile_pool(name="dram", bufs=1, space="DRAM"))

        wq_sb = [consts.tile([128, H * DK], F16, name=f"wq{t}") for t in range(4)]
        wk_sb = [consts.tile([128, H * DK], F16, name=f"wk{t}") for t in range(4)]
        xqT_sb = [consts.tile([128, S], F16, name=f"xqT{t}") for t in range(4)]
        xkT_sb = [consts.tile([128, S], F16, name=f"xkT{t}") for t in range(4)]
        m01T_sb = [consts.tile([128, S], F16, name=f"m01T_{t}") for t in range(NQT)]
        v_sb = [consts.tile([128, H * DK], F16, name=f"v{t}") for t in range(NQT)]
        ones_sb = consts.tile([128, 1], F16)
        nc.vector.memset(ones_sb, 1.0)

        for t in range(4):
            nc.sync.dma_start(wq_sb[t], wq.ap()[t * 128 : (t + 1) * 128, :])
            nc.sync.dma_start(wk_sb[t], wk.ap()[t * 128 : (t + 1) * 128, :])
            nc.sync.dma_start(xqT_sb[t], xqT.ap()[t * 128 : (t + 1) * 128, :])
            nc.sync.dma_start(xkT_sb[t], xkT.ap()[t * 128 : (t + 1) * 128, :])
        for t in range(NQT):
            nc.sync.dma_start(m01T_sb[t], m01T.ap()[t * 128 : (t + 1) * 128, :])

        craw_d = dram.tile([H, 128, S], F16)
        rec32_d = dram.tile([H, S], F32)

        # ---- V projection ----
        with (
            tc.tile_pool(name="p0", bufs=1) as p0,
            tc.tile_pool(name="p0ps", bufs=2, space="PSUM") as p0ps,
        ):
            xvT_sb = [p0.tile([128, S], F16, name=f"xvT{t}") for t in range(4)]
            wv_sb = [p0.tile([128, H * DK], F16, name=f"wv{t}") for t in range(4)]
            for t in range(4):
                nc.sync.dma_start(xvT_sb[t], xvT.ap()[t * 128 : (t + 1) * 128, :])
                nc.sync.dma_start(wv_sb[t], wv.ap()[t * 128 : (t + 1) * 128, :])
            for st in range(NQT):
                vps = p0ps.tile([128, H * DK], F32, tag="vps")
                for nch in range(4):
                    for kt in range(4):
                        nc.tensor.matmul(
                            vps[:, nch * 512 : (nch + 1) * 512],
                            xvT_sb[kt][:, st * 128 : (st + 1) * 128],
                            wv_sb[kt][:, nch * 512 : (nch + 1) * 512],
                            start=(kt == 0),
                            stop=(kt == 3),
                        )
                nc.scalar.copy(v_sb[st], vps)

        # ---- head loop ----
        with (
            tc.tile_pool(name="hd", bufs=2) as hd,
            tc.tile_pool(name="work", bufs=3) as work,
            tc.tile_pool(name="etmp", bufs=10) as etmp,
            tc.tile_pool(name="scps", bufs=2, space="PSUM") as scps,
            tc.tile_pool(name="crps", bufs=2, space="PSUM") as crps,
            tc.tile_pool(name="onps", bufs=1, space="PSUM") as onps,
        ):
            for h in range(H):
                hsl = slice(h * DK, (h + 1) * DK)

                # projections qT_h / kT_h
                qT_h = hd.tile([128, S], F16, name="qT_h")
                kT_h = hd.tile([128, S], F16, name="kT_h")
                for dst, w_sb, x_sb in ((qT_h, wq_sb, xqT_sb), (kT_h, wk_sb, xkT_sb)):
                    for qc in range(2):
                        pps = scps.tile([128, 512], F32, tag="sc")
                        for kt in range(4):
                            nc.tensor.matmul(
                                pps,
                                w_sb[kt][:, hsl],
                                x_sb[kt][:, qc * 512 : (qc + 1) * 512],
                                start=(kt == 0),
                                stop=(kt == 3),
                            )
                        nc.scalar.copy(dst[:, qc * 512 : (qc + 1) * 512], pps)

                # k-major pass: scores^T, exp, mask, context + row sums
                cps = crps.tile([128, S], F32, tag="cr")
                onp = onps.tile([1, S], F32, tag="on")
                etms = []
                for kt in range(NQT):
                    etm = etmp.tile([128, S], F16, name="etm_t")
                    for qc in range(2):
                        qsl = slice(qc * 512, (qc + 1) * 512)
                        sps = scps.tile([128, 512], F32, tag="sc")
                        nc.tensor.matmul(
                            sps,
                            kT_h[:, kt * 128 : (kt + 1) * 128],
                            qT_h[:, qsl],
                            start=True,
                            stop=True,
                        )
                        et = work.tile([128, 512], F16, name="et_t")
                        nc.scalar.activation(et, sps, AF.Exp)
                        nc.vector.tensor_mul(
                            etm[:, qsl], et, m01T_sb[kt][:, qsl]
                        )
                        nc.tensor.matmul(
                            cps[:, qsl],
                            v_sb[kt][:, hsl],
                            etm[:, qsl],
                            start=(kt == 0),
                            stop=(kt == NQT - 1),
                        )
                        nc.tensor.matmul(
                            onp[0:1, qsl],
                            ones_sb,
                            etm[:, qsl],
                            start=(kt == 0),
                            stop=(kt == NQT - 1),
                        )
                    etms.append(etm)

                # row reciprocals: 1/s = exp(-ln(s)) on ACT (same table set
                # as Exp), written q-linear, then broadcast across partitions
                lnr = work.tile([1, S], F32, name="lnr")
                nc.scalar.activation(lnr, onp, AF.Ln)
                recips_row = work.tile([1, S], F32, name="recips_row")
                nc.scalar.activation(recips_row, lnr, AF.Exp, scale=-1.0)
                r32row = rec32_d[h : h + 1]
                nc.sync.dma_start(
                    bass.AP(
                        tensor=r32row.tensor,
                        offset=r32row.offset,
                        ap=[[1, 1], [1, S]],
                    ),
                    recips_row,
                )
                rbc32 = work.tile([128, S], F32, name="rbc32")
                nc.sync.dma_start(rbc32, _bcast_ap(r32row, 128, [[1, S]]))
                rbc16 = work.tile([128, S], F16, name="rbc16")
                nc.vector.tensor_copy(rbc16, rbc32)

                # attn^T tiles out
                for kt in range(NQT):
                    p_t = work.tile([128, S], F16, name="p_t")
                    nc.vector.tensor_mul(p_t, etms[kt], rbc16)
                    nc.sync.dma_start(
                        attnT.ap()[h, kt * 128 : (kt + 1) * 128, :], p_t
                    )

                # scale context rows and stash
                craws_t = work.tile([128, S], F16, name="craws_t")
                nc.vector.tensor_mul(craws_t, cps, rbc32)
                nc.sync.dma_start(craw_d[h], craws_t)

        # ---- fc + residual + layernorm ----
        with (
            tc.tile_pool(name="fc", bufs=1) as fc,
            tc.tile_pool(name="fcw", bufs=3) as fcw,
            tc.tile_pool(name="fcps", bufs=1, space="PSUM") as fcps,
        ):
            wfc_sb = [fc.tile([128, DM], F16, name=f"wfc{t}") for t in range(H)]
            for t in range(H):
                nc.sync.dma_start(wfc_sb[t], wfc.ap()[t * 128 : (t + 1) * 128, :])
            eps_t = fc.tile([128, 1], F32)
            nc.vector.memset(eps_t, EPS)

            ops = [
                fcps.tile([128, DM], F32, name=f"ops{qt}", tag=f"o{qt}")
                for qt in range(NQT)
            ]
            for kt2 in range(H):
                craw_t = fcw.tile([128, S], F16, name="craw_t")
                nc.sync.dma_start(craw_t, craw_d[kt2])
                for qt in range(NQT):
                    nc.tensor.matmul(
                        ops[qt],
                        craw_t[:, qt * 128 : (qt + 1) * 128],
                        wfc_sb[kt2],
                        start=(kt2 == 0),
                        stop=(kt2 == H - 1),
                    )
            for qt in range(NQT):
                xq_t = fcw.tile([128, DM], F32, name="xq_t")
                nc.sync.dma_start(xq_t, xq.ap()[qt * 128 : (qt + 1) * 128, :])
                o1 = fcw.tile([128, DM], F32, name="o1")
                nc.vector.tensor_add(o1, ops[qt], xq_t)
                stats = fcw.tile([128, 6], F32, name="stats")
                nc.vector.bn_stats(stats, o1)
                mv = fcw.tile([128, 2], F32, name="mv")
                nc.vector.bn_aggr(mv, stats)
                std = fcw.tile([128, 1], F32, name="std")
                nc.scalar.activation(
                    std, mv[:, 1:2], AF.Sqrt, bias=eps_t, scale=1.0
                )
                nc.vector.reciprocal(std, std)
                out_t = fcw.tile([128, DM], F32, name="out_t")
                nc.vector.tensor_scalar(
                    out=out_t,
                    in0=o1,
                    scalar1=mv[:, 0:1],
                    scalar2=std,
                    op0=ALU.subtract,
                    op1=ALU.mult,
                )
                nc.sync.dma_start(outp.ap()[qt * 128 : (qt + 1) * 128, :], out_t)


_CACHED = {}


def _get_kernel():
    if "nc" not in _CACHED:
        _CACHED["nc"] = build_kernel(8)
    return _CACHED["nc"]


def _prep_core_inputs(b, input_Q, input_K, input_V, attn_mask, W_Q, W_K, W_V, W_fc):
    f16 = np.float16
    scale = np.float32(1.0 / np.sqrt(DK))
    m01T = (~attn_mask[b]).T.astype(f16)
    return {
        "xqT": np.ascontiguousarray(input_Q[b].T).astype(f16),
        "xkT": np.ascontiguousarray(input_K[b].T).astype(f16),
        "xvT": np.ascontiguousarray(input_V[b].T).astype(f16),
        "xq": np.ascontiguousarray(input_Q[b]).astype(np.float32),
        "wq": (W_Q * scale).astype(f16),
        "wk": W_K.astype(f16),
        "wv": W_V.astype(f16),
        "wfc": W_fc.astype(f16),
        "m01T": np.ascontiguousarray(m01T),
    }


def kernel(input_Q, input_K, input_V, attn_mask, W_Q, W_K, W_V, W_fc, _trace=False):
    input_Q = np.asarray(input_Q, dtype=np.float32)
    input_K = np.asarray(input_K, dtype=np.float32)
    input_V = np.asarray(input_V, dtype=np.float32)
    attn_mask = np.asarray(attn_mask, dtype=bool)
    W_Q = np.asarray(W_Q, dtype=np.float32)
    W_K = np.asarray(W_K, dtype=np.float32)
    W_V = np.asarray(W_V, dtype=np.float32)
    W_fc = np.asarray(W_fc, dtype=np.float32)

    B = input_Q.shape[0]
    assert B == 8

    nc = _get_kernel()
    in_maps = [
        _prep_core_inputs(b, input_Q, input_K, input_V, attn_mask, W_Q, W_K, W_V, W_fc)
        for b in range(B)
    ]
    res = bass_utils.run_bass_kernel_spmd(
        nc, in_maps, core_ids=list(range(B)), trace=_trace
    )
    out = np.stack([res.results[b]["outp"] for b in range(B)])
    # unshard: per-core attn^T (H, S_k, S_q) fp16 -> full attn (B, H, S_q, S_k) f32
    attn = np.empty((B, H, S, S), dtype=np.float32)
    for b in range(B):
        at = res.results[b]["attnT"].astype(np.float32)
        attn[b] = at.swapaxes(-1, -2)
    if _trace:
        _CACHED["last_result"] = res
    return out, attn


# revision 16
# speedup vs baseline: 1.0150x; 1.0150x over previous
"""Multi-head attention + residual + LayerNorm TRN2 Bass kernel.

Problem: B=8, S=1024, d_model=512, 16 heads x d_k=128.
Returns (out, attn) like the reference:
    out  (8, 1024, 512)  f32   layernorm(context @ W_fc + input_Q)
    attn (8, 16, 1024, 1024) f32  softmax probabilities

Sharding: data-parallel over batch, one batch element per NeuronCore (8 cores).

Per-core design (fp16 matmul inputs / f32 PSUM accumulate), k-major only:
  - host pre-transposes X inputs to (512,1024) fp16; 1/sqrt(128) folded into W_Q
  - V = X_v @ W_V once in (seq, 2048) layout
  - per head h: qT_h/kT_h = (W^T X^T) tiles (d_k on partitions)
    for each k-tile: S^T = K_kt Q^T -> exp -> mask-mul -> E^T_m, which feeds
      * context matmuls C^T_h += V_kt^T E^T_m (PSUM accumulation)
      * ones-matmuls: row sums = 1^T E^T_m (PSUM accumulation, M=1)
      * after reciprocals arrive: P^T = E^T_m * (1/rowsum) -> attnT out (fp16)
    reciprocal broadcast across partitions via a tiny DRAM round-trip
    C^T_h scaled by 1/rowsum and stashed to DRAM as fp16
  - fc: O = C^T.T @ W_fc over all 16 head blocks + residual, then LayerNorm
  - host: attn = attnT.swapaxes(-1,-2).astype(f32)  (part of unsharding)
"""

import os
import sys

for _p in ("/opt/trn_rl_repo", "/root/.axon_site/_ro/trn_rl_repo"):
    if os.path.isdir(_p) and _p not in sys.path:
        sys.path.append(_p)

import numpy as np

import concourse.bass as bass
import concourse.bacc as bacc
import concourse.tile as tile
import concourse.mybir as mybir
from concourse import bass_utils

F32 = mybir.dt.float32
F16 = mybir.dt.float16
AF = mybir.ActivationFunctionType
ALU = mybir.AluOpType

S = 1024
DM = 512
H = 16
DK = 128
NQT = S // 128  # 8 k/q tiles of 128
EPS = 1e-5


def build_kernel(n_cores: int = 8):
    nc = bacc.Bacc(
        "TRN2",
        target_bir_lowering=False,
        debug=False,
        enable_asserts=False,
        num_devices=n_cores,
    )

    xqT = nc.dram_tensor("xqT", (DM, S), F16, kind="ExternalInput")
    xkT = nc.dram_tensor("xkT", (DM, S), F16, kind="ExternalInput")
    xvT = nc.dram_tensor("xvT", (DM, S), F16, kind="ExternalInput")
    xq = nc.dram_tensor("xq", (S, DM), F32, kind="ExternalInput")
    wq = nc.dram_tensor("wq", (DM, H * DK), F16, kind="ExternalInput")
    wk = nc.dram_tensor("wk", (DM, H * DK), F16, kind="ExternalInput")
    wv = nc.dram_tensor("wv", (DM, H * DK), F16, kind="ExternalInput")
    wfc = nc.dram_tensor("wfc", (H * DK, DM), F16, kind="ExternalInput")
    m01T = nc.dram_tensor("m01T", (S, S), F16, kind="ExternalInput")

    attnT = nc.dram_tensor("attnT", (H, S, S), F16, kind="ExternalOutput")
    outp = nc.dram_tensor("outp", (S, DM), F32, kind="ExternalOutput")

    with tile.TileContext(nc) as tc:
        _body(nc, tc, xqT, xkT, xvT, xq, wq, wk, wv, wfc, m01T, attnT, outp)

    nc.compile()
    return nc


def _bcast_ap(row_ap, parts, free_dims):
    """Partition-broadcast read AP over a DRAM row."""
    return bass.AP(
        tensor=row_ap.tensor,
        offset=row_ap.offset,
        ap=[[0, parts]] + free_dims,
    )


def _body(nc, tc, xqT, xkT, xvT, xq, wq, wk, wv, wfc, m01T, attnT, outp):
    from contextlib import ExitStack

    with ExitStack() as ctx:
        consts = ctx.enter_context(tc.tile_pool(name="consts", bufs=1))
        dram = ctx.enter_context(tc.tile_pool(name="dram", bufs=1, space="DRAM"))

        wq_sb = [consts.tile([128, H * DK], F16, name=f"wq{t}") for t in range(4)]
        wk_sb = [consts.tile([128, H * DK], F16, name=f"wk{t}") for t in range(4)]
        xqT_sb = [consts.tile([128, S], F16, name=f"xqT{t}") for t in range(4)]
        xkT_sb = [consts.tile([128, S], F16, name=f"xkT{t}") for t in range(4)]
        m01T_sb = [consts.tile([128, S], F16, name=f"m01T_{t}") for t in range(NQT)]
        v_sb = [consts.tile([128, H * DK], F16, name=f"v{t}") for t in range(NQT)]
        ones_sb = consts.tile([128, 1], F16)
        nc.vector.memset(ones_sb, 1.0)
        from concourse.masks import make_identity

        ident = consts.tile([128, 128], F32)
        make_identity(nc, ident)

        for t in range(4):
            nc.sync.dma_start(wq_sb[t], wq.ap()[t * 128 : (t + 1) * 128, :])
            nc.sync.dma_start(wk_sb[t], wk.ap()[t * 128 : (t + 1) * 128, :])
            nc.sync.dma_start(xqT_sb[t], xqT.ap()[t * 128 : (t + 1) * 128, :])
            nc.sync.dma_start(xkT_sb[t], xkT.ap()[t * 128 : (t + 1) * 128, :])
        for t in range(NQT):
            nc.sync.dma_start(m01T_sb[t], m01T.ap()[t * 128 : (t + 1) * 128, :])

        craw_d = dram.tile([H, 128, S], F16)
        rec32_d = dram.tile([H, S], F32)

        # ---- V projection ----
        with (
            tc.tile_pool(name="p0", bufs=1) as p0,
            tc.tile_pool(name="p0ps", bufs=2, space="PSUM") as p0ps,
        ):
            xvT_sb = [p0.tile([128, S], F16, name=f"xvT{t}") for t in range(4)]
            wv_sb = [p0.tile([128, H * DK], F16, name=f"wv{t}") for t in range(4)]
            for t in range(4):
                nc.sync.dma_start(xvT_sb[t], xvT.ap()[t * 128 : (t + 1) * 128, :])
                nc.sync.dma_start(wv_sb[t], wv.ap()[t * 128 : (t + 1) * 128, :])
            for st in range(NQT):
                vps = p0ps.tile([128, H * DK], F32, tag="vps")
                for nch in range(4):
                    for kt in range(4):
                        nc.tensor.matmul(
                            vps[:, nch * 512 : (nch + 1) * 512],
                            xvT_sb[kt][:, st * 128 : (st + 1) * 128],
                            wv_sb[kt][:, nch * 512 : (nch + 1) * 512],
                            start=(kt == 0),
                            stop=(kt == 3),
                        )
                nc.scalar.copy(v_sb[st], vps)

        # ---- head loop ----
        with (
            tc.tile_pool(name="hd", bufs=2) as hd,
            tc.tile_pool(name="work", bufs=3) as work,
            tc.tile_pool(name="etmp", bufs=12) as etmp,
            tc.tile_pool(name="scps", bufs=4, space="PSUM") as scps,
            tc.tile_pool(name="crps", bufs=1, space="PSUM") as crps,
            tc.tile_pool(name="onps", bufs=1, space="PSUM") as onps,
        ):
            for h in range(H):
                hsl = slice(h * DK, (h + 1) * DK)

                # projections qT_h / kT_h
                qT_h = hd.tile([128, S], F16, name="qT_h")
                kT_h = hd.tile([128, S], F16, name="kT_h")
                for dst, w_sb, x_sb in ((qT_h, wq_sb, xqT_sb), (kT_h, wk_sb, xkT_sb)):
                    for qc in range(2):
                        pps = scps.tile([128, 512], F32, tag="sc")
                        for kt in range(4):
                            nc.tensor.matmul(
                                pps,
                                w_sb[kt][:, hsl],
                                x_sb[kt][:, qc * 512 : (qc + 1) * 512],
                                start=(kt == 0),
                                stop=(kt == 3),
                            )
                        nc.scalar.copy(dst[:, qc * 512 : (qc + 1) * 512], pps)

                # k-major pass: scores^T, exp, mask, context + row sums
                cps = crps.tile([128, S], F32, tag="cr")
                onp = onps.tile([1, S], F32, tag="on")
                etms = []
                for kt in range(NQT):
                    etm = etmp.tile([128, S], F16, name="etm_t")
                    for qc in range(2):
                        qsl = slice(qc * 512, (qc + 1) * 512)
                        sps = scps.tile([128, 512], F32, tag="sc")
                        nc.tensor.matmul(
                            sps,
                            kT_h[:, kt * 128 : (kt + 1) * 128],
                            qT_h[:, qsl],
                            start=True,
                            stop=True,
                        )
                        et = work.tile([128, 512], F16, name="et_t")
                        nc.scalar.activation(et, sps, AF.Exp)
                        nc.vector.tensor_mul(
                            etm[:, qsl], et, m01T_sb[kt][:, qsl]
                        )
                        nc.tensor.matmul(
                            cps[:, qsl],
                            v_sb[kt][:, hsl],
                            etm[:, qsl],
                            start=(kt == 0),
                            stop=(kt == NQT - 1),
                        )
                        nc.tensor.matmul(
                            onp[0:1, qsl],
                            ones_sb,
                            etm[:, qsl],
                            start=(kt == 0),
                            stop=(kt == NQT - 1),
                        )
                    etms.append(etm)

                # drain context psum unscaled right away (frees the bank)
                craw_sb = work.tile([128, S], F16, name="craw_sb")
                nc.vector.tensor_copy(craw_sb, cps)

                # row reciprocals: sums (1,S) -> 8x PE transpose -> (128,8)
                # -> DVE reciprocal -> PE transpose back -> (8,128) -> DRAM
                # q-linear -> partition-broadcast read
                sums_row = work.tile([1, S], F32, name="sums_row")
                nc.scalar.copy(sums_row, onp)
                sumsT = scps.tile([128, NQT], F32, tag="sc")
                for t in range(NQT):
                    nc.tensor.transpose(
                        sumsT[:, t : t + 1],
                        sums_row[0:1, t * 128 : (t + 1) * 128],
                        ident[:1, :1],
                    )
                recips = work.tile([128, NQT], F32, name="recips")
                nc.vector.reciprocal(recips, sumsT)
                recipsT_ps = scps.tile([NQT, 128], F32, tag="sc")
                nc.tensor.transpose(recipsT_ps, recips, ident)
                recipsT = work.tile([NQT, 128], F32, name="recipsT")
                nc.vector.tensor_copy(recipsT, recipsT_ps)
                r32row = rec32_d[h : h + 1]
                nc.sync.dma_start(
                    bass.AP(
                        tensor=r32row.tensor,
                        offset=r32row.offset,
                        ap=[[128, NQT], [1, 128]],
                    ),
                    recipsT,
                )
                rbc32 = work.tile([128, S], F32, name="rbc32")
                nc.sync.dma_start(rbc32, _bcast_ap(r32row, 128, [[1, S]]))
                rbc16 = work.tile([128, S], F16, name="rbc16")
                nc.scalar.copy(rbc16, rbc32)

                # attn^T tiles out
                for kt in range(NQT):
                    p_t = work.tile([128, S], F16, name="p_t")
                    nc.vector.tensor_mul(p_t, etms[kt], rbc16)
                    nc.sync.dma_start(
                        attnT.ap()[h, kt * 128 : (kt + 1) * 128, :], p_t
                    )

                # scale context rows by 1/rowsum and stash
                craws_t = work.tile([128, S], F16, name="craws_t")
                nc.vector.tensor_mul(craws_t, craw_sb, rbc32)
                nc.sync.dma_start(craw_d[h], craws_t)

        # ---- fc + residual + layernorm ----
        with (
            tc.tile_pool(name="fc", bufs=1) as fc,
            tc.tile_pool(name="fcw", bufs=3) as fcw,
            tc.tile_pool(name="fcps", bufs=1, space="PSUM") as fcps,
        ):
            wfc_sb = [fc.tile([128, DM], F16, name=f"wfc{t}") for t in range(H)]
            for t in range(H):
                nc.sync.dma_start(wfc_sb[t], wfc.ap()[t * 128 : (t + 1) * 128, :])
            eps_t = fc.tile([128, 1], F32)
            nc.vector.memset(eps_t, EPS)

            ops = [
                fcps.tile([128, DM], F32, name=f"ops{qt}", tag=f"o{qt}")
                for qt in range(NQT)
            ]
            for kt2 in range(H):
                craw_t = fcw.tile([128, S], F16, name="craw_t")
                nc.sync.dma_start(craw_t, craw_d[kt2])
                for qt in range(NQT):
                    nc.tensor.matmul(
                        ops[qt],
                        craw_t[:, qt * 128 : (qt + 1) * 128],
                        wfc_sb[kt2],
                        start=(kt2 == 0),
                        stop=(kt2 == H - 1),
                    )
            for qt in range(NQT):
                xq_t = fcw.tile([128, DM], F32, name="xq_t")
                nc.sync.dma_start(xq_t, xq.ap()[qt * 128 : (qt + 1) * 128, :])
                o1 = fcw.tile([128, DM], F32, name="o1")
                nc.vector.tensor_add(o1, ops[qt], xq_t)
                stats = fcw.tile([128, 6], F32, name="stats")
                nc.vector.bn_stats(stats, o1)
                mv = fcw.tile([128, 2], F32, name="mv")
                nc.vector.bn_aggr(mv, stats)
                std = fcw.tile([128, 1], F32, name="std")
                nc.scalar.activation(
                    std, mv[:, 1:2], AF.Sqrt, bias=eps_t, scale=1.0
                )
                nc.vector.reciprocal(std, std)
                out_t = fcw.tile([128, DM], F32, name="out_t")
                nc.vector.tensor_scalar(
                    out=out_t,
                    in0=o1,
                    scalar1=mv[:, 0:1],
                    scalar2=std,
                    op0=ALU.subtract,
                    op1=ALU.mult,
                )
                nc.sync.dma_start(outp.ap()[qt * 128 : (qt + 1) * 128, :], out_t)


_CACHED = {}


def _get_kernel():
    if "nc" not in _CACHED:
        _CACHED["nc"] = build_kernel(8)
    return _CACHED["nc"]


def _prep_core_inputs(b, input_Q, input_K, input_V, attn_mask, W_Q, W_K, W_V, W_fc):
    f16 = np.float16
    scale = np.float32(1.0 / np.sqrt(DK))
    m01T = (~attn_mask[b]).T.astype(f16)
    return {
        "xqT": np.ascontiguousarray(input_Q[b].T).astype(f16),
        "xkT": np.ascontiguousarray(input_K[b].T).astype(f16),
        "xvT": np.ascontiguousarray(input_V[b].T).astype(f16),
        "xq": np.ascontiguousarray(input_Q[b]).astype(np.float32),
        "wq": (W_Q * scale).astype(f16),
        "wk": W_K.astype(f16),
        "wv": W_V.astype(f16),
        "wfc": W_fc.astype(f16),
        "m01T": np.ascontiguousarray(m01T),
    }


def kernel(input_Q, input_K, input_V, attn_mask, W_Q, W_K, W_V, W_fc, _trace=False):
    input_Q = np.asarray(input_Q, dtype=np.float32)
    input_K = np.asarray(input_K, dtype=np.float32)
    input_V = np.asarray(input_V, dtype=np.float32)
    attn_mask = np.asarray(attn_mask, dtype=bool)
    W_Q = np.asarray(W_Q, dtype=np.float32)
    W_K = np.asarray(W_K, dtype=np.float32)
    W_V = np.asarray(W_V, dtype=np.float32)
    W_fc = np.asarray(W_fc, dtype=np.float32)

    B = input_Q.shape[0]
    assert B == 8

    nc = _get_kernel()
    in_maps = [
        _prep_core_inputs(b, input_Q, input_K, input_V, attn_mask, W_Q, W_K, W_V, W_fc)
        for b in range(B)
    ]
    res = bass_utils.run_bass_kernel_spmd(
        nc, in_maps, core_ids=list(range(B)), trace=_trace
    )
    out = np.stack([res.results[b]["outp"] for b in range(B)])
    # unshard: per-core attn^T (H, S_k, S_q) fp16 -> full attn (B, H, S_q, S_k) f32
    attn = np.empty((B, H, S, S), dtype=np.float32)
    for b in range(B):
        at = res.results[b]["attnT"].astype(np.float32)
        attn[b] = at.swapaxes(-1, -2)
    if _trace:
        _CACHED["last_result"] = res
    return out, attn
